# revision 17
# baseline (speedup 1.0000x reference)
"""Trainium2 Bass kernel for nn_Attention_31267361915369 (v5).

Computation (per batch example, T=4096, D=1024):
    h   = tanh(x @ W1.T + b1)          # [T, D]
    s   = h @ w2.T + b2                # [T]
    e   = exp(s)                       # num/den is exactly shift-invariant
    num = cumsum(e * x, axis=0)        # [T, D]
    den = cumsum(e)                    # [T]
    out = tanh([num/den, x] @ Wc.T + bc)

Restructure: cumsum over T commutes with the D-side matmul, so with
A = Wc[:, :D], B = Wc[:, D:]:
    out = tanh(cumsum(e * y)/den + z),  y = x @ A.T, z = x @ B.T

v5 design (hw-calibrated: MM(512,bf16) ~280ns, fp8 DR ~308ns, ACT
[128,1024] ~1.2-1.3us, DVE TT f32 ~1.5us, DVE from PSUM ~1.9us):
  - score GEMM in fp8-e4m3 DoubleRow (8 instrs vs 16; fp8 noise only
    perturbs softmax scores; end-to-end rel err ~3e-3 vs 2e-2 budget).
  - 3-stage software pipeline: front(i) = ph/py GEMMs + score chain;
    cumsum(i-1) = pd/pc tri-matmuls + u1; combine(i-2) = u+tanh+store.
    The cumsum matmuls for tile i-1 are emitted AFTER front(i)'s GEMMs,
    so the in-order PE queue always has ready work while the
    cross-engine score chain (tanh->prod->reduce->exp->ey) of tile i
    completes; pz(i-1) is emitted last and consumed from PSUM by the
    combine add at the next iteration's head (no z copy).
  - running carry rides in the f32 PSUM cumsum itself: row 127 of
    pc/pd is staged (ACT) -> DMA'd to partition 0 -> added into
    ey(i+1) row 0 / e(i+1) row 0 on DVE, one full tile-time of slack.
  - score reduction (prod = h*w2, row-sum) runs on the otherwise-idle
    GPSIMD engine; ey/u1 scaling on ACT with per-partition scale APs.
  - output stored bf16, widened on host.

Distribution: data-parallel over batch B=8 across 8 NeuronCores, weights
replicated, no collectives.
"""

import os
import sys

if "/opt/trn_rl_repo" not in sys.path:
    sys.path.insert(0, "/opt/trn_rl_repo")

from contextlib import ExitStack

import ml_dtypes
import numpy as np

import concourse.bass as bass
import concourse.tile as tile
from concourse import bacc, mybir
from concourse.bass_utils import run_bass_kernel_spmd

P = 128
D = 1024
T_FULL = 4096
N_CORES = 8
NK = D // P         # bf16 k-chunks
NKK = D // (2 * P)  # fp8 DoubleRow k-passes (K=256 each)

BF = mybir.dt.bfloat16
F32 = mybir.dt.float32
FP8 = mybir.dt.float8e4
AFT = mybir.ActivationFunctionType
PM = mybir.MatmulPerfMode

_BUILD_CACHE: dict = {}

USE_FP8 = True    # DoubleRow fp8 for the score GEMM
USE_POOL = True   # score prod/reduce on GPSIMD
K_NO_STORE = os.environ.get("K_NO_STORE") == "1"  # timing probe only


def build(T: int = T_FULL, use_b1: bool = False, use_bc: bool = False,
          repeat: int = 1):
    key = (T, use_b1, use_bc, repeat)
    if key in _BUILD_CACHE:
        return _BUILD_CACHE[key]

    assert T % P == 0
    NT = T // P

    nc = bacc.Bacc("TRN2", target_bir_lowering=False, debug=False)

    # host-pretransposed x: xt[i, p, k, t] = x[i*128+t, k*128+p]
    xt_d = nc.declare_dram_parameter("xt", [T * NK, P], BF, isOutput=False)
    w18_d = nc.declare_dram_parameter("w18", [NKK * P * 2, D], FP8, isOutput=False)
    wa8_d = nc.declare_dram_parameter("wa8", [NKK * P * 2, D], FP8, isOutput=False)
    w1t_d = nc.declare_dram_parameter("w1t", [D, D], BF, isOutput=False)
    wat_d = nc.declare_dram_parameter("wat", [D, D], BF, isOutput=False)
    wbt_d = nc.declare_dram_parameter("wbt", [D, D], BF, isOutput=False)
    w2r_d = nc.declare_dram_parameter("w2r", [P, D], F32, isOutput=False)
    tri_d = nc.declare_dram_parameter("tri", [P, P], BF, isOutput=False)
    b1_d = nc.declare_dram_parameter("b1r", [1, D], BF, isOutput=False) if use_b1 else None
    bc_d = nc.declare_dram_parameter("bcr", [1, D], BF, isOutput=False) if use_bc else None
    out_d = nc.declare_dram_parameter("out", [T, D], BF, isOutput=True)

    xt_t = xt_d.ap().rearrange("(n p k) q -> n p k q", p=P, k=NK)
    w18_t = w18_d.ap().rearrange("(kk p j) e -> kk p j e", p=P, j=2)
    wa8_t = wa8_d.ap().rearrange("(kk p j) e -> kk p j e", p=P, j=2)
    w1_t = w1t_d.ap().rearrange("(k p) e -> k p e", p=P)
    wa_t = wat_d.ap().rearrange("(k p) e -> k p e", p=P)
    wb_t = wbt_d.ap().rearrange("(k p) e -> k p e", p=P)
    out_t = out_d.ap().rearrange("(n p) d -> n p d", p=P)

    with tile.TileContext(nc) as tc, ExitStack() as ctx:
        consts = ctx.enter_context(tc.tile_pool(name="consts", bufs=1))
        xtp = ctx.enter_context(tc.tile_pool(name="xtp", bufs=4))
        x8p = ctx.enter_context(tc.tile_pool(name="x8p", bufs=4))
        hp = ctx.enter_context(tc.tile_pool(name="hp", bufs=2))
        prodp = ctx.enter_context(tc.tile_pool(name="prodp", bufs=2))
        eyp = ctx.enter_context(tc.tile_pool(name="eyp", bufs=2))
        u1p = ctx.enter_context(tc.tile_pool(name="u1p", bufs=2))
        up = ctx.enter_context(tc.tile_pool(name="up", bufs=2))
        outp = ctx.enter_context(tc.tile_pool(name="outp", bufs=2))
        colp = ctx.enter_context(tc.tile_pool(name="colp", bufs=8))
        cstp = ctx.enter_context(tc.tile_pool(name="cstp", bufs=2))
        crowp = ctx.enter_context(tc.tile_pool(name="crowp", bufs=2))
        # PSUM budget (8 banks): php hosts the two [128,512] score halves
        # plus pd (same-shape slot, col 0 used) on 2 bufs = 2 banks; pmm
        # rotates {pc, py, pz} on 3 bufs = 6 banks.
        php = ctx.enter_context(tc.tile_pool(name="php", bufs=2, space="PSUM"))
        pmm = ctx.enter_context(tc.tile_pool(name="pmm", bufs=3, space="PSUM"))

        tri_sb = consts.tile([P, P], BF, tag="tri")
        nc.sync.dma_start(out=tri_sb[:], in_=tri_d.ap())
        # f32: wide bf16 TensorTensor/TensorReduce DVE ops hang on this hw
        w2r_sb = consts.tile([P, D], F32, tag="w2r")
        nc.sync.dma_start(out=w2r_sb[:], in_=w2r_d.ap())
        if use_b1:
            b1_sb = consts.tile([1, D], BF, tag="b1")
            nc.sync.dma_start(out=b1_sb[:], in_=b1_d.ap())
        if use_bc:
            bc_sb = consts.tile([1, D], BF, tag="bc")
            nc.sync.dma_start(out=bc_sb[:], in_=bc_d.ap())
        w18_sb, wa8_sb, w1_sb, wa_sb, wb_sb = [], [], [], [], []
        if USE_FP8:
            for kk in range(NKK):
                t8 = consts.tile([P, 2, D], FP8, tag=f"w18_{kk}")
                nc.sync.dma_start(out=t8[:], in_=w18_t[kk])
                w18_sb.append(t8)
            for kk in range(NKK):
                t8 = consts.tile([P, 2, D], FP8, tag=f"wa8_{kk}")
                nc.sync.dma_start(out=t8[:], in_=wa8_t[kk])
                wa8_sb.append(t8)
        else:
            for k in range(NK):
                t1 = consts.tile([P, D], BF, tag=f"w1_{k}")
                nc.sync.dma_start(out=t1[:], in_=w1_t[k])
                w1_sb.append(t1)
        for k in range(NK):
            ta = consts.tile([P, D], BF, tag=f"wa_{k}")
            nc.sync.dma_start(out=ta[:], in_=wa_t[k])
            wa_sb.append(ta)
        for k in range(NK):
            tb = consts.tile([P, D], BF, tag=f"wb_{k}")
            nc.sync.dma_start(out=tb[:], in_=wb_t[k])
            wb_sb.append(tb)

        state = {}
        C2 = [(0, 512), (512, 1024)]

        def load_x(i):
            xT = xtp.tile([P, NK, P], BF, tag="xt")
            nc.sync.dma_start(out=xT[:], in_=xt_t[i])
            state[("xt", i)] = xT

        def cast_x8(i):
            # fp8 DoubleRow operand = elementwise cast of xt (same k order);
            # runs on the otherwise-idle GPSIMD engine.
            xT = state[("xt", i)]
            x8 = x8p.tile([P, NK, P], FP8, tag="x8")
            nc.gpsimd.tensor_copy(x8[:], xT[:])
            state[("x8", i)] = x8

        def gemm(dst, lhs_tiles, rhs_sb, bias_sb):
            nkc = len(rhs_sb)
            for k in range(nkc):
                last = k == nkc - 1 and bias_sb is None
                for lo, hi in C2:
                    nc.tensor.matmul(
                        dst[:, lo:hi], lhs_tiles(k), rhs_sb[k][:, lo:hi],
                        start=(k == 0), stop=last,
                    )
            if bias_sb is not None:
                for lo, hi in C2:
                    nc.tensor.matmul(
                        dst[:, lo:hi], tri_sb[0:1, :], bias_sb[0:1, lo:hi],
                        start=False, stop=True,
                    )

        def front_ph(i):
            """Score GEMM in two [128,512] half-tiles + tanh + prod."""
            h_sb = hp.tile([P, D], F32, tag="h")
            for ci, (lo, hi) in enumerate(C2):
                ph = php.tile([P, 512], F32, tag="ph", name=f"ph{ci}")
                if USE_FP8:
                    x8 = state[("x8", i)]
                    for kk in range(NKK):
                        last = kk == NKK - 1 and not use_b1
                        nc.tensor.matmul(
                            ph[:], x8[:, 2 * kk:2 * kk + 2, :],
                            w18_sb[kk][:, :, lo:hi],
                            start=(kk == 0), stop=last,
                            perf_mode=PM.DoubleRow,
                        )
                    if use_b1:
                        nc.tensor.matmul(ph[:], tri_sb[0:1, :],
                                         b1_sb[0:1, lo:hi], start=False, stop=True)
                else:
                    xT = state[("xt", i)]
                    for k in range(NK):
                        last = k == NK - 1 and not use_b1
                        nc.tensor.matmul(ph[:], xT[:, k, :],
                                         w1_sb[k][:, lo:hi],
                                         start=(k == 0), stop=last)
                    if use_b1:
                        nc.tensor.matmul(ph[:], tri_sb[0:1, :],
                                         b1_sb[0:1, lo:hi], start=False, stop=True)
                nc.scalar.activation(h_sb[:, lo:hi], ph[:], AFT.Tanh)

            eng = nc.gpsimd if USE_POOL else nc.vector
            prod = prodp.tile([P, D], F32, tag="prod")
            eng.tensor_mul(prod[:], h_sb[:], w2r_sb[:])
            state[("prod", i)] = prod

        def front_score(i):
            """Row-sum + exp + e*y for tile i (py must be emitted already)."""
            prod = state.pop(("prod", i))
            py = state.pop(("py", i))
            s_col = colp.tile([P, 1], F32, tag="s", name="s_col")
            nc.vector.reduce_sum(s_col[:], prod[:], axis=mybir.AxisListType.X)
            e_col = colp.tile([P, 1], F32, tag="e", name="e_col")
            nc.scalar.activation(e_col[:], s_col[:], AFT.Exp)
            e_bf = colp.tile([P, 1], BF, tag="ebf", name="e_bf")
            nc.scalar.copy(e_bf[:], e_col[:])
            ey = eyp.tile([P, D], BF, tag="ey")
            nc.vector.tensor_scalar_mul(ey[:], py[:], e_col[:])
            state[("eye", i)] = (ey, e_bf)

        def y_gemm(i):
            py = pmm.tile([P, D], F32, tag="mm", name="py")
            if USE_FP8:
                x8 = state.pop(("x8", i))
                for lo, hi in C2:
                    for kk in range(NKK):
                        nc.tensor.matmul(
                            py[:, lo:hi], x8[:, 2 * kk:2 * kk + 2, :],
                            wa8_sb[kk][:, :, lo:hi],
                            start=(kk == 0), stop=(kk == NKK - 1),
                            perf_mode=PM.DoubleRow,
                        )
            else:
                xT = state[("xt", i)]
                gemm(py, lambda k: xT[:, k, :], wa_sb, None)
            state[("py", i)] = py

        def carry_in(i):
            """Fold the running totals (through tile i-1) into tile i's
            ey row 0 / e row 0; tri row 0 is all ones, so the tri matmul
            propagates them to every output row."""
            ey, e_bf = state[("eye", i)]
            crow = state.pop(("crow", i - 1))
            nc.vector.tensor_add(ey[0:1, :], ey[0:1, :], crow[0:1, 0:D])
            nc.vector.tensor_add(e_bf[0:1, :], e_bf[0:1, :], crow[0:1, D:D + 1])

        def cumsum(i):
            """pd/pc tri-matmuls + u1 + totals staging for tile i."""
            ey, e_bf = state.pop(("eye", i))
            # pd rides a same-shape slot in the php pool; col 0 is the result
            pdt = php.tile([P, 512], F32, tag="ph", name="pd")
            pd = pdt[:, 0:1]
            nc.tensor.matmul(pd, tri_sb[:], e_bf[:], start=True, stop=True)
            pc = pmm.tile([P, D], F32, tag="mm", name="pc")
            for lo, hi in C2:
                nc.tensor.matmul(pc[:, lo:hi], tri_sb[:], ey[:, lo:hi],
                                 start=True, stop=True)
            rden = colp.tile([P, 1], F32, tag="rden", name="rden")
            nc.vector.reciprocal(rden[:], pd)
            u1 = u1p.tile([P, D], F32, tag="u1")
            nc.scalar.mul(u1[:], pc[:], rden[:])
            if i < NT - 1:
                cst = cstp.tile([P, D + 1], BF, tag="cst")
                nc.scalar.copy(cst[96:128, 0:D], pc[96:128, :])
                nc.scalar.copy(cst[96:128, D:D + 1], pdt[96:128, 0:1])
                crow = crowp.tile([1, D + 1], BF, tag="crow")
                nc.sync.dma_start(out=crow[0:1, :], in_=cst[127:128, :])
                state[("crow", i)] = crow
            state[("u1", i)] = u1

        def z_gemm(i):
            xT_prev = state.pop(("xt", i))
            pz = pmm.tile([P, D], F32, tag="mm", name="pz")
            gemm(pz, lambda k: xT_prev[:, k, :], wb_sb, bc_sb if use_bc else None)
            state[("pz", i)] = pz

        def combine_u(i):
            """u = u1 + pz (DVE, ready at iteration head)."""
            u1 = state.pop(("u1", i))
            pz = state.pop(("pz", i))
            u_sb = up.tile([P, D], BF, tag="u")
            nc.vector.tensor_add(u_sb[:], u1[:], pz[:])
            state[("u", i)] = u_sb

        def combine_out(i):
            u_sb = state.pop(("u", i))
            o_sb = outp.tile([P, D], BF, tag="out")
            nc.scalar.activation(o_sb[:], u_sb[:], AFT.Tanh)
            if not K_NO_STORE:
                nc.sync.dma_start(out=out_t[i], in_=o_sb[:])

        def whole_pipeline():
            state.clear()
            load_x(0)
            load_x(1)
            if USE_FP8:
                cast_x8(0)
            for i in range(NT + 2):
                a, b, c = i, i - 1, i - 2
                if a + 2 < NT:
                    load_x(a + 2)
                if c >= 0:
                    combine_u(c)       # DVE head; frees pz(c)'s PSUM slot
                if a < NT:
                    front_ph(a)        # PE ph halves; ACT tanh halves; prod
                if c >= 0:
                    combine_out(c)     # ACT tanh + store, after tanh halves
                if 0 <= b < NT:
                    cumsum(b)          # PE pd+pc (inputs ready since iter b)
                if a < NT:
                    y_gemm(a)          # PE 16 MMs
                    if USE_FP8 and a + 1 < NT:
                        cast_x8(a + 1)  # Pool, behind prod(a) in the FIFO
                    front_score(a)     # DVE reduce; ACT exp/ey
                if 0 <= b < NT:
                    z_gemm(b)          # PE 16 MMs, consumed next iter head
                if 1 <= a < NT:
                    carry_in(a)        # DVE row-0 carry adds (after ey_mul)

        if repeat == 1:
            whole_pipeline()
        else:
            with tc.For_i(0, repeat, 1):
                whole_pipeline()

    nc.compile()
    _BUILD_CACHE[key] = nc
    return nc


def _bf16(a):
    return np.ascontiguousarray(np.asarray(a, dtype=np.float32)).astype(
        ml_dtypes.bfloat16
    )


def _fp8(a):
    return np.ascontiguousarray(np.asarray(a, dtype=np.float32)).astype(
        ml_dtypes.float8_e4m3
    )


def make_in_maps(x, W1, b1, w2, b2, Wc, bc, T=T_FULL):
    """Host-side prep: shard x over batch, pre-transpose/replicate weights."""
    x = np.asarray(x, dtype=np.float32)
    W1 = np.asarray(W1, dtype=np.float32)
    Wc = np.asarray(Wc, dtype=np.float32)
    w2 = np.asarray(w2, dtype=np.float32).reshape(1, -1)
    b1 = np.asarray(b1, dtype=np.float32)
    bc = np.asarray(bc, dtype=np.float32)
    use_b1 = bool(np.any(b1 != 0.0))
    use_bc = bool(np.any(bc != 0.0))
    # b2 shifts every score equally; exp(b2) cancels in num/den.

    w1t = _bf16(W1.T)
    # fp8 DoubleRow weights: w18[kk, p, j, e] = W1[e, 256kk + 128j + p]
    w18 = np.ascontiguousarray(
        _fp8(W1.T).reshape(NKK, 2, P, D).transpose(0, 2, 1, 3)
    ).reshape(NKK * P * 2, D)
    wa8 = np.ascontiguousarray(
        _fp8(Wc[:, :D].T).reshape(NKK, 2, P, D).transpose(0, 2, 1, 3)
    ).reshape(NKK * P * 2, D)
    wat = _bf16(Wc[:, :D].T)
    wbt = _bf16(Wc[:, D:].T)
    w2r = np.ascontiguousarray(np.broadcast_to(w2, (P, D)).astype(np.float32))
    tri = _bf16(np.triu(np.ones((P, P), np.float32)))

    NT = T // P
    in_maps = []
    for i in range(x.shape[0]):
        xb = _bf16(x[i, :T, :])
        # xt[i, p, k, t] = x[i*128+t, k*128+p], 2KB-contiguous per partition
        xt = np.ascontiguousarray(
            xb.reshape(NT, P, NK, P).transpose(0, 3, 2, 1)
        ).reshape(T * NK, P)
        m = {
            "xt": xt,
            "w18": w18,
            "wa8": wa8,
            "w1t": w1t,
            "wat": wat,
            "wbt": wbt,
            "w2r": w2r,
            "tri": tri,
        }
        if use_b1:
            m["b1r"] = _bf16(b1.reshape(1, D))
        if use_bc:
            m["bcr"] = _bf16(bc.reshape(1, D))
        in_maps.append(m)
    return in_maps, use_b1, use_bc


def kernel(x, W1, b1, w2, b2, Wc, bc):
    in_maps, use_b1, use_bc = make_in_maps(x, W1, b1, w2, b2, Wc, bc)
    nc = build(T_FULL, use_b1, use_bc)
    res = run_bass_kernel_spmd(nc, in_maps, core_ids=list(range(N_CORES)))
    out = np.stack(
        [np.asarray(res.results[i]["out"]).astype(np.float32) for i in range(N_CORES)],
        axis=0,
    )
    return out


# revision 20
# speedup vs baseline: 1.0515x; 1.0515x over previous
"""Trainium2 Bass kernel for nn_Attention_31267361915369 (v5).

Computation (per batch example, T=4096, D=1024):
    h   = tanh(x @ W1.T + b1)          # [T, D]
    s   = h @ w2.T + b2                # [T]
    e   = exp(s)                       # num/den is exactly shift-invariant
    num = cumsum(e * x, axis=0)        # [T, D]
    den = cumsum(e)                    # [T]
    out = tanh([num/den, x] @ Wc.T + bc)

Restructure: cumsum over T commutes with the D-side matmul, so with
A = Wc[:, :D], B = Wc[:, D:]:
    out = tanh(cumsum(e * y)/den + z),  y = x @ A.T, z = x @ B.T

v5 design (hw-calibrated: MM(512,bf16) ~280ns, fp8 DR ~308ns, ACT
[128,1024] ~1.2-1.3us, DVE TT f32 ~1.5us, DVE from PSUM ~1.9us):
  - score GEMM in fp8-e4m3 DoubleRow (8 instrs vs 16; fp8 noise only
    perturbs softmax scores; end-to-end rel err ~3e-3 vs 2e-2 budget).
  - 3-stage software pipeline: front(i) = ph/py GEMMs + score chain;
    cumsum(i-1) = pd/pc tri-matmuls + u1; combine(i-2) = u+tanh+store.
    The cumsum matmuls for tile i-1 are emitted AFTER front(i)'s GEMMs,
    so the in-order PE queue always has ready work while the
    cross-engine score chain (tanh->prod->reduce->exp->ey) of tile i
    completes; pz(i-1) is emitted last and consumed from PSUM by the
    combine add at the next iteration's head (no z copy).
  - running carry rides in the f32 PSUM cumsum itself: row 127 of
    pc/pd is staged (ACT) -> DMA'd to partition 0 -> added into
    ey(i+1) row 0 / e(i+1) row 0 on DVE, one full tile-time of slack.
  - score reduction (prod = h*w2, row-sum) runs on the otherwise-idle
    GPSIMD engine; ey/u1 scaling on ACT with per-partition scale APs.
  - output stored bf16, widened on host.

Distribution: data-parallel over batch B=8 across 8 NeuronCores, weights
replicated, no collectives.
"""

import os
import sys

if "/opt/trn_rl_repo" not in sys.path:
    sys.path.insert(0, "/opt/trn_rl_repo")

from contextlib import ExitStack

import ml_dtypes
import numpy as np

import concourse.bass as bass
import concourse.tile as tile
from concourse import bacc, mybir
from concourse.bass_utils import run_bass_kernel_spmd

P = 128
D = 1024
T_FULL = 4096
N_CORES = 8
NK = D // P         # bf16 k-chunks
NKK = D // (2 * P)  # fp8 DoubleRow k-passes (K=256 each)

BF = mybir.dt.bfloat16
F32 = mybir.dt.float32
FP8 = mybir.dt.float8e4
AFT = mybir.ActivationFunctionType
PM = mybir.MatmulPerfMode

_BUILD_CACHE: dict = {}

USE_FP8 = True    # DoubleRow fp8 for the score GEMM
USE_POOL = True   # score prod/reduce on GPSIMD
K_NO_STORE = os.environ.get("K_NO_STORE") == "1"  # timing probe only


def build(T: int = T_FULL, use_b1: bool = False, use_bc: bool = False,
          repeat: int = 1):
    key = (T, use_b1, use_bc, repeat)
    if key in _BUILD_CACHE:
        return _BUILD_CACHE[key]

    assert T % P == 0
    NT = T // P

    nc = bacc.Bacc("TRN2", target_bir_lowering=False, debug=False)

    # host-pretransposed x: xt[i, p, k, t] = x[i*128+t, k*128+p]
    xt_d = nc.declare_dram_parameter("xt", [T * NK, P], BF, isOutput=False)
    w18_d = nc.declare_dram_parameter("w18", [NKK * P * 2, D], FP8, isOutput=False)
    wa8_d = nc.declare_dram_parameter("wa8", [NKK * P * 2, D], FP8, isOutput=False)
    w1t_d = nc.declare_dram_parameter("w1t", [D, D], BF, isOutput=False)
    wat_d = nc.declare_dram_parameter("wat", [D, D], BF, isOutput=False)
    wbt_d = nc.declare_dram_parameter("wbt", [D, D], BF, isOutput=False)
    w2r_d = nc.declare_dram_parameter("w2r", [P, D], F32, isOutput=False)
    tri_d = nc.declare_dram_parameter("tri", [P, P], BF, isOutput=False)
    b1_d = nc.declare_dram_parameter("b1r", [1, D], BF, isOutput=False) if use_b1 else None
    bc_d = nc.declare_dram_parameter("bcr", [1, D], BF, isOutput=False) if use_bc else None
    out_d = nc.declare_dram_parameter("out", [T, D], BF, isOutput=True)

    xt_t = xt_d.ap().rearrange("(n p k) q -> n p k q", p=P, k=NK)
    w18_t = w18_d.ap().rearrange("(kk p j) e -> kk p j e", p=P, j=2)
    wa8_t = wa8_d.ap().rearrange("(kk p j) e -> kk p j e", p=P, j=2)
    w1_t = w1t_d.ap().rearrange("(k p) e -> k p e", p=P)
    wa_t = wat_d.ap().rearrange("(k p) e -> k p e", p=P)
    wb_t = wbt_d.ap().rearrange("(k p) e -> k p e", p=P)
    out_t = out_d.ap().rearrange("(n p) d -> n p d", p=P)

    with tile.TileContext(nc) as tc, ExitStack() as ctx:
        consts = ctx.enter_context(tc.tile_pool(name="consts", bufs=1))
        xtp = ctx.enter_context(tc.tile_pool(name="xtp", bufs=4))
        x8p = ctx.enter_context(tc.tile_pool(name="x8p", bufs=4))
        hp = ctx.enter_context(tc.tile_pool(name="hp", bufs=2))
        prodp = ctx.enter_context(tc.tile_pool(name="prodp", bufs=2))
        eyp = ctx.enter_context(tc.tile_pool(name="eyp", bufs=2))
        u1p = ctx.enter_context(tc.tile_pool(name="u1p", bufs=2))
        up = ctx.enter_context(tc.tile_pool(name="up", bufs=2))
        outp = ctx.enter_context(tc.tile_pool(name="outp", bufs=2))
        colp = ctx.enter_context(tc.tile_pool(name="colp", bufs=8))
        cstp = ctx.enter_context(tc.tile_pool(name="cstp", bufs=2))
        crowp = ctx.enter_context(tc.tile_pool(name="crowp", bufs=2))
        # PSUM budget (8 banks): php hosts the two [128,512] score halves
        # plus pd (same-shape slot, col 0 used) on 2 bufs = 2 banks; pmm
        # rotates {pc, py, pz} on 3 bufs = 6 banks.
        php = ctx.enter_context(tc.tile_pool(name="php", bufs=2, space="PSUM"))
        pmm = ctx.enter_context(tc.tile_pool(name="pmm", bufs=3, space="PSUM"))

        tri_sb = consts.tile([P, P], BF, tag="tri")
        nc.sync.dma_start(out=tri_sb[:], in_=tri_d.ap())
        # f32: wide bf16 TensorTensor/TensorReduce DVE ops hang on this hw
        w2r_sb = consts.tile([P, D], F32, tag="w2r")
        nc.sync.dma_start(out=w2r_sb[:], in_=w2r_d.ap())
        if use_b1:
            b1_sb = consts.tile([1, D], BF, tag="b1")
            nc.sync.dma_start(out=b1_sb[:], in_=b1_d.ap())
        if use_bc:
            bc_sb = consts.tile([1, D], BF, tag="bc")
            nc.sync.dma_start(out=bc_sb[:], in_=bc_d.ap())
        w18_sb, wa8_sb, w1_sb, wa_sb, wb_sb = [], [], [], [], []
        if USE_FP8:
            for kk in range(NKK):
                t8 = consts.tile([P, 2, D], FP8, tag=f"w18_{kk}")
                nc.sync.dma_start(out=t8[:], in_=w18_t[kk])
                w18_sb.append(t8)
            for kk in range(NKK):
                t8 = consts.tile([P, 2, D], FP8, tag=f"wa8_{kk}")
                nc.sync.dma_start(out=t8[:], in_=wa8_t[kk])
                wa8_sb.append(t8)
        else:
            for k in range(NK):
                t1 = consts.tile([P, D], BF, tag=f"w1_{k}")
                nc.sync.dma_start(out=t1[:], in_=w1_t[k])
                w1_sb.append(t1)
        for k in range(NK):
            ta = consts.tile([P, D], BF, tag=f"wa_{k}")
            nc.sync.dma_start(out=ta[:], in_=wa_t[k])
            wa_sb.append(ta)
        for k in range(NK):
            tb = consts.tile([P, D], BF, tag=f"wb_{k}")
            nc.sync.dma_start(out=tb[:], in_=wb_t[k])
            wb_sb.append(tb)

        state = {}
        C2 = [(0, 512), (512, 1024)]

        def load_x(i):
            xT = xtp.tile([P, NK, P], BF, tag="xt")
            nc.sync.dma_start(out=xT[:], in_=xt_t[i])
            state[("xt", i)] = xT

        def cast_x8(i):
            # fp8 DoubleRow operand = elementwise cast of xt (same k order);
            # runs on the otherwise-idle GPSIMD engine.
            xT = state[("xt", i)]
            x8 = x8p.tile([P, NK, P], FP8, tag="x8")
            nc.gpsimd.tensor_copy(x8[:], xT[:])
            state[("x8", i)] = x8

        def gemm(dst, lhs_tiles, rhs_sb, bias_sb):
            nkc = len(rhs_sb)
            for k in range(nkc):
                last = k == nkc - 1 and bias_sb is None
                for lo, hi in C2:
                    nc.tensor.matmul(
                        dst[:, lo:hi], lhs_tiles(k), rhs_sb[k][:, lo:hi],
                        start=(k == 0), stop=last,
                    )
            if bias_sb is not None:
                for lo, hi in C2:
                    nc.tensor.matmul(
                        dst[:, lo:hi], tri_sb[0:1, :], bias_sb[0:1, lo:hi],
                        start=False, stop=True,
                    )

        def front_ph(i):
            """Score GEMM in two [128,512] half-tiles + tanh + prod."""
            h_sb = hp.tile([P, D], F32, tag="h")
            for ci, (lo, hi) in enumerate(C2):
                ph = php.tile([P, 512], F32, tag="ph", name=f"ph{ci}")
                if USE_FP8:
                    x8 = state[("x8", i)]
                    for kk in range(NKK):
                        last = kk == NKK - 1 and not use_b1
                        nc.tensor.matmul(
                            ph[:], x8[:, 2 * kk:2 * kk + 2, :],
                            w18_sb[kk][:, :, lo:hi],
                            start=(kk == 0), stop=last,
                            perf_mode=PM.DoubleRow,
                        )
                    if use_b1:
                        nc.tensor.matmul(ph[:], tri_sb[0:1, :],
                                         b1_sb[0:1, lo:hi], start=False, stop=True)
                else:
                    xT = state[("xt", i)]
                    for k in range(NK):
                        last = k == NK - 1 and not use_b1
                        nc.tensor.matmul(ph[:], xT[:, k, :],
                                         w1_sb[k][:, lo:hi],
                                         start=(k == 0), stop=last)
                    if use_b1:
                        nc.tensor.matmul(ph[:], tri_sb[0:1, :],
                                         b1_sb[0:1, lo:hi], start=False, stop=True)
                nc.scalar.activation(h_sb[:, lo:hi], ph[:], AFT.Tanh)

            eng = nc.gpsimd if USE_POOL else nc.vector
            prod = prodp.tile([P, D], F32, tag="prod")
            eng.tensor_mul(prod[:], h_sb[:], w2r_sb[:])
            state[("prod", i)] = prod

        def front_score(i):
            """Row-sum + exp + e*y for tile i (py must be emitted already)."""
            prod = state.pop(("prod", i))
            py = state.pop(("py", i))
            s_col = colp.tile([P, 1], F32, tag="s", name="s_col")
            nc.vector.reduce_sum(s_col[:], prod[:], axis=mybir.AxisListType.X)
            e_col = colp.tile([P, 1], F32, tag="e", name="e_col")
            nc.scalar.activation(e_col[:], s_col[:], AFT.Exp)
            e_bf = colp.tile([P, 1], BF, tag="ebf", name="e_bf")
            nc.scalar.copy(e_bf[:], e_col[:])
            ey = eyp.tile([P, D], BF, tag="ey")
            nc.scalar.mul(ey[:], py[:], e_col[:])
            state[("eye", i)] = (ey, e_bf)

        def y_gemm(i):
            py = pmm.tile([P, D], F32, tag="mm", name="py")
            if USE_FP8:
                x8 = state.pop(("x8", i))
                for lo, hi in C2:
                    for kk in range(NKK):
                        nc.tensor.matmul(
                            py[:, lo:hi], x8[:, 2 * kk:2 * kk + 2, :],
                            wa8_sb[kk][:, :, lo:hi],
                            start=(kk == 0), stop=(kk == NKK - 1),
                            perf_mode=PM.DoubleRow,
                        )
            else:
                xT = state[("xt", i)]
                gemm(py, lambda k: xT[:, k, :], wa_sb, None)
            state[("py", i)] = py

        def carry_in(i):
            """Fold the running totals (through tile i-1) into tile i's
            ey row 0 / e row 0; tri row 0 is all ones, so the tri matmul
            propagates them to every output row."""
            ey, e_bf = state[("eye", i)]
            crow = state.pop(("crow", i - 1))
            nc.vector.tensor_add(ey[0:1, :], ey[0:1, :], crow[0:1, 0:D])
            nc.vector.tensor_add(e_bf[0:1, :], e_bf[0:1, :], crow[0:1, D:D + 1])

        def cumsum(i):
            """pd/pc tri-matmuls + u1 + totals staging for tile i."""
            ey, e_bf = state.pop(("eye", i))
            # pd rides a same-shape slot in the php pool; col 0 is the result
            pdt = php.tile([P, 512], F32, tag="ph", name="pd")
            pd = pdt[:, 0:1]
            nc.tensor.matmul(pd, tri_sb[:], e_bf[:], start=True, stop=True)
            pc = pmm.tile([P, D], F32, tag="mm", name="pc")
            for lo, hi in C2:
                nc.tensor.matmul(pc[:, lo:hi], tri_sb[:], ey[:, lo:hi],
                                 start=True, stop=True)
            rden = colp.tile([P, 1], F32, tag="rden", name="rden")
            nc.vector.reciprocal(rden[:], pd)
            u1 = u1p.tile([P, D], F32, tag="u1")
            nc.scalar.mul(u1[:], pc[:], rden[:])
            if i < NT - 1:
                cst = cstp.tile([P, D + 1], BF, tag="cst")
                nc.scalar.copy(cst[96:128, 0:D], pc[96:128, :])
                nc.scalar.copy(cst[96:128, D:D + 1], pdt[96:128, 0:1])
                crow = crowp.tile([1, D + 1], BF, tag="crow")
                nc.sync.dma_start(out=crow[0:1, :], in_=cst[127:128, :])
                state[("crow", i)] = crow
            state[("u1", i)] = u1

        def z_gemm(i):
            xT_prev = state.pop(("xt", i))
            pz = pmm.tile([P, D], F32, tag="mm", name="pz")
            gemm(pz, lambda k: xT_prev[:, k, :], wb_sb, bc_sb if use_bc else None)
            state[("pz", i)] = pz

        def combine_u(i):
            """u = u1 + pz (DVE, ready at iteration head)."""
            u1 = state.pop(("u1", i))
            pz = state.pop(("pz", i))
            u_sb = up.tile([P, D], BF, tag="u")
            nc.vector.tensor_add(u_sb[:], u1[:], pz[:])
            state[("u", i)] = u_sb

        def combine_out(i):
            u_sb = state.pop(("u", i))
            o_sb = outp.tile([P, D], BF, tag="out")
            nc.scalar.activation(o_sb[:], u_sb[:], AFT.Tanh)
            if not K_NO_STORE:
                nc.sync.dma_start(out=out_t[i], in_=o_sb[:])

        def whole_pipeline():
            state.clear()
            load_x(0)
            load_x(1)
            if USE_FP8:
                cast_x8(0)
            for i in range(NT + 2):
                a, b, c = i, i - 1, i - 2
                if a + 2 < NT:
                    load_x(a + 2)
                if c >= 0:
                    combine_u(c)       # DVE head; frees pz(c)'s PSUM slot
                if a < NT:
                    front_ph(a)        # PE ph halves; ACT tanh halves; prod
                if c >= 0:
                    combine_out(c)     # ACT tanh + store, after tanh halves
                if 0 <= b < NT:
                    cumsum(b)          # PE pd+pc (inputs ready since iter b)
                if a < NT:
                    y_gemm(a)          # PE 16 MMs
                    if USE_FP8 and a + 1 < NT:
                        cast_x8(a + 1)  # Pool, behind prod(a) in the FIFO
                    front_score(a)     # DVE reduce; ACT exp/ey
                if 0 <= b < NT:
                    z_gemm(b)          # PE 16 MMs, consumed next iter head
                if 1 <= a < NT:
                    carry_in(a)        # DVE row-0 carry adds (after ey_mul)

        if repeat == 1:
            whole_pipeline()
        else:
            with tc.For_i(0, repeat, 1):
                whole_pipeline()

    nc.compile()
    _BUILD_CACHE[key] = nc
    return nc


def _bf16(a):
    return np.ascontiguousarray(np.asarray(a, dtype=np.float32)).astype(
        ml_dtypes.bfloat16
    )


def _fp8(a):
    return np.ascontiguousarray(np.asarray(a, dtype=np.float32)).astype(
        ml_dtypes.float8_e4m3
    )


def make_in_maps(x, W1, b1, w2, b2, Wc, bc, T=T_FULL):
    """Host-side prep: shard x over batch, pre-transpose/replicate weights."""
    x = np.asarray(x, dtype=np.float32)
    W1 = np.asarray(W1, dtype=np.float32)
    Wc = np.asarray(Wc, dtype=np.float32)
    w2 = np.asarray(w2, dtype=np.float32).reshape(1, -1)
    b1 = np.asarray(b1, dtype=np.float32)
    bc = np.asarray(bc, dtype=np.float32)
    use_b1 = bool(np.any(b1 != 0.0))
    use_bc = bool(np.any(bc != 0.0))
    # b2 shifts every score equally; exp(b2) cancels in num/den.

    w1t = _bf16(W1.T)
    # fp8 DoubleRow weights: w18[kk, p, j, e] = W1[e, 256kk + 128j + p]
    w18 = np.ascontiguousarray(
        _fp8(W1.T).reshape(NKK, 2, P, D).transpose(0, 2, 1, 3)
    ).reshape(NKK * P * 2, D)
    wa8 = np.ascontiguousarray(
        _fp8(Wc[:, :D].T).reshape(NKK, 2, P, D).transpose(0, 2, 1, 3)
    ).reshape(NKK * P * 2, D)
    wat = _bf16(Wc[:, :D].T)
    wbt = _bf16(Wc[:, D:].T)
    w2r = np.ascontiguousarray(np.broadcast_to(w2, (P, D)).astype(np.float32))
    tri = _bf16(np.triu(np.ones((P, P), np.float32)))

    NT = T // P
    in_maps = []
    for i in range(x.shape[0]):
        xb = _bf16(x[i, :T, :])
        # xt[i, p, k, t] = x[i*128+t, k*128+p], 2KB-contiguous per partition
        xt = np.ascontiguousarray(
            xb.reshape(NT, P, NK, P).transpose(0, 3, 2, 1)
        ).reshape(T * NK, P)
        m = {
            "xt": xt,
            "w18": w18,
            "wa8": wa8,
            "w1t": w1t,
            "wat": wat,
            "wbt": wbt,
            "w2r": w2r,
            "tri": tri,
        }
        if use_b1:
            m["b1r"] = _bf16(b1.reshape(1, D))
        if use_bc:
            m["bcr"] = _bf16(bc.reshape(1, D))
        in_maps.append(m)
    return in_maps, use_b1, use_bc


def kernel(x, W1, b1, w2, b2, Wc, bc):
    in_maps, use_b1, use_bc = make_in_maps(x, W1, b1, w2, b2, Wc, bc)
    nc = build(T_FULL, use_b1, use_bc)
    res = run_bass_kernel_spmd(nc, in_maps, core_ids=list(range(N_CORES)))
    out = np.stack(
        [np.asarray(res.results[i]["out"]).astype(np.float32) for i in range(N_CORES)],
        axis=0,
    )
    return out
